# revision 21
# baseline (speedup 1.0000x reference)
"""Trainium2 Bass kernel for Llama GQA self-attention (b=2, s=2048, E=4096,
32 Q heads / 8 KV heads, RoPE, causal) sharded tensor-parallel over 8
NeuronCores (one KV-head group per core).

v3: TB=512 phase-1 blocks (half the PE instruction count) with attention
outputs aliased onto the dead q-head tiles to pay for the bigger x
double-buffer; work-queue scheduler (pe_q/dve_q) that pumps phase-1
leftovers / phase-3 groups between each AV chain and its dependent
transpose (covers DVE recip/scale latency) and at j-starts (covers exp
latency); RoPE emitted in 512-col chunks through the queue; b0-h3 and
b1-h0 interleaved at the j level; startup DMAs split across 4 queues;
output stores split in halves on alternating queues.

Per-core pipeline:
  phase 1: qkv_sb[6][128, 4096] (bf16, SBUF) = w_qkvT.T @ xT
  phase 2: per (batch, head): scoresT[kt, qt] = kT.T @ qT -> exp
           (ScalarE, paired [128,1024] tiles) -> tri-mask diag (DVE) ->
           AV: out[qt, 129] = expT.T @ [v | 1] -> normalize -> transpose
           -> aoT bf16 (written into the dead q slots of qkv_sb)
  phase 3: out[tok, 4096] = sum_et aoT[et].T @ w_outT  (bf16, half-row
           stores interleaved into b1 attention)
Host: sum the 8 partial outputs.
"""

import os
import sys
from collections import deque

import numpy as np

for _p in ("/opt/trn_rl_repo", "/root/.axon_site/_ro/trn_rl_repo"):
    if os.path.isdir(_p) and _p not in sys.path:
        sys.path.append(_p)

import ml_dtypes  # noqa: E402

import concourse.bass as bass  # noqa: E402
import concourse.mybir as mybir  # noqa: E402
import concourse.tile as tile  # noqa: E402
from concourse import bacc  # noqa: E402
from concourse.bass_utils import run_bass_kernel_spmd  # noqa: E402

F32 = mybir.dt.float32
BF16 = mybir.dt.bfloat16
NPBF16 = ml_dtypes.bfloat16

EMBED = 4096
N_HEADS = 32
N_KV = 8
HEAD_DIM = 128
Q_PER_KV = 4
B = 2
S = 2048
TOK = B * S            # 4096
NCORES = 8
ODIM = 768             # per-core qkv rows: 4 q heads + k + v
SCALE = HEAD_DIM ** -0.5
ROPE_BASE = 10000.0

TB = 512               # phase-1 token block (moving N)
NB = TOK // TB         # 8
ET1 = EMBED // 128     # 32 contraction tiles
NM1 = ODIM // 128      # 6 output row tiles

ALU = mybir.AluOpType
ACTF = mybir.ActivationFunctionType


def _emit(nc, tc, h):
    with (
        tc.tile_pool(name="cp", bufs=1) as cp,
        tc.tile_pool(name="psum", bufs=1, space="PSUM") as pp,
        tc.tile_pool(name="ep", bufs=1) as ep,
    ):
        tri = cp.tile([128, 128], BF16, tag="tri")
        idbf = cp.tile([128, 128], BF16, tag="idbf")
        cos_t = cp.tile([128, S], BF16, tag="cos")
        sin_t = cp.tile([128, S], BF16, tag="sin")

        qkv = [
            cp.tile([128, TOK], BF16, tag=f"qkv{m}", name=f"qkv{m}")
            for m in range(NM1)
        ]
        # attention outputs overwrite the q-head slots: column ranges are
        # only written after scores consumed them (WAR handled by tile fw)
        aoT = qkv

        vaug = [[None] * 16 for _ in range(B)]
        wo = [None] * 8

        # ---------------- work-queue scheduler --------------------------
        pe_q = deque()
        dve_q = deque()
        pe_rate = [1.0]
        cred = [0.0]

        def set_rate(r):
            pe_rate[0] = r
            cred[0] = 0.0

        def pump():
            cred[0] += pe_rate[0]
            while pe_q and cred[0] >= 1.0:
                cred[0] -= 1.0
                pe_q.popleft()()
            if dve_q:
                dve_q.popleft()()

        def vaug_one(b, vt):
            # v slice is bf16 in SBUF already: transpose tokens<->dims
            pst = pp.tile([128, 128], BF16, tag="op", bufs=4)
            nc.tensor.matmul(
                pst,
                lhsT=qkv[5][:, b * S + vt * 128 : b * S + (vt + 1) * 128],
                rhs=idbf,
                is_transpose=True,
            )
            va = ep.tile([128, 132], BF16, tag="vaug", bufs=33)
            nc.gpsimd.memset(va[:, 128:129], 1.0)
            nc.vector.tensor_copy(va[:, 0:128], pst)
            vaug[b][vt] = va

        def rope_chunk(b, hs, c):
            # half-layout RoPE, in place, 512-col chunk c: rows 0:64
            # pair-elem x1, rows 64:128 x2; rt = [x2*-s; x1*+s];
            # src = src*cos + rt
            lo = b * S + c * 512
            sl = qkv[hs][:, lo : lo + 512]
            rt = ep.tile([128, 512], BF16, tag="rt", bufs=2)
            nc.vector.tensor_copy(rt[0:64, :], sl[64:128, :])
            nc.vector.tensor_copy(rt[64:128, :], sl[0:64, :])
            nc.vector.tensor_mul(rt, rt, sin_t[:, c * 512 : (c + 1) * 512])
            nc.vector.tensor_mul(sl, sl, cos_t[:, c * 512 : (c + 1) * 512])
            nc.vector.tensor_tensor(sl, sl, rt, ALU.add)

        def rope_chunks(b, hs):
            return [
                (lambda bb=b, hh=hs, cc=c: rope_chunk(bb, hh, cc))
                for c in range(4)
            ]

        def head_j(b, hh, j, u_cb=None):
            kr = qkv[4][:, b * S : (b + 1) * S]
            qr = qkv[hh][:, b * S : (b + 1) * S]
            nt = 4 * j + 4
            ets = {}
            for p0 in range(0, nt, 2):
                ps2 = pp.tile([128, 1024], F32, tag="sc2", bufs=2)
                lo = None
                for t in (p0, p0 + 1):
                    off = (t - p0) * 512
                    c0 = 128 * (t - 4 * j) if t >= 4 * j else 0
                    if lo is None:
                        lo = off + c0
                    nc.tensor.matmul(
                        ps2[:, off + c0 : off + 512],
                        lhsT=kr[:, t * 128 : (t + 1) * 128],
                        rhs=qr[:, j * 512 + c0 : (j + 1) * 512],
                        start=True,
                        stop=True,
                    )
                et2 = ep.tile([128, 1024], BF16, tag="exp", bufs=12)
                if p0 + 1 >= 4 * j:
                    # diag pair: written ranges are disjoint; exp
                    # each segment to avoid uninitialized psum
                    for t in (p0, p0 + 1):
                        off = (t - p0) * 512
                        c0 = 128 * (t - 4 * j) if t >= 4 * j else 0
                        nc.scalar.activation(
                            et2[:, off + c0 : off + 512],
                            ps2[:, off + c0 : off + 512],
                            ACTF.Exp, scale=SCALE,
                        )
                else:
                    nc.scalar.activation(
                        et2[:, lo:1024], ps2[:, lo:1024], ACTF.Exp,
                        scale=SCALE,
                    )
                for t in (p0, p0 + 1):
                    if t >= 4 * j:
                        off = (t - p0) * 512
                        c0 = 128 * (t - 4 * j)
                        nc.vector.tensor_mul(
                            et2[:, off + c0 : off + c0 + 128],
                            et2[:, off + c0 : off + c0 + 128],
                            tri,
                        )
                ets[p0] = et2
            if u_cb is not None:
                u_cb()  # cover first-exp latency before the AV chain
            for u in range(4):
                nkt = 4 * j + u + 1
                av = pp.tile([128, 512], F32, tag="op", bufs=4)
                for t in range(nkt):
                    sl = ets[t - t % 2][
                        :, (t % 2) * 512 + u * 128 : (t % 2) * 512 + (u + 1) * 128
                    ]
                    nc.tensor.matmul(
                        av[:, 0:129],
                        lhsT=sl,
                        rhs=vaug[b][t][:, 0:129],
                        start=(t == 0),
                        stop=(t == nkt - 1),
                    )
                rec = ep.tile([128, 1], F32, tag="rec", bufs=2)
                nc.vector.reciprocal(rec, av[:, 128:129])
                ao = ep.tile([128, 128], BF16, tag="ao", bufs=2)
                nc.vector.tensor_scalar_mul(ao, av[:, 0:128], rec)
                if u_cb is not None:
                    u_cb()  # PE filler covers the recip/scale latency
                pst = pp.tile([128, 128], BF16, tag="op", bufs=4)
                nc.tensor.matmul(pst, lhsT=ao, rhs=idbf, is_transpose=True)
                tok0 = b * S + j * 512 + u * 128
                nc.vector.tensor_copy(aoT[hh][:, tok0 : tok0 + 128], pst)



        # ---------------- phase 1: qkv projection (SBUF-resident) -------
        with tc.tile_pool(name="p1", bufs=1) as p1:
            wqm = []
            for m in range(NM1):
                w_ = p1.tile([128, ET1, 128], BF16, tag=f"wq{m}", name=f"wq{m}")
                wqm.append(w_)
            xb0 = p1.tile([128, ET1, TB], BF16, tag="xb", bufs=2)
            # chunked just-in-time startup loads on 3 queues, ordered to
            # match block-0's m-interleaved consumption; the first x
            # chunk is split so the first matmuls start sooner
            # first-consumed pieces lead each queue
            nc.scalar.dma_start(wqm[0][:, 0:8, :], h["wqkvT"][0][:, 0:8, :])
            nc.gpsimd.dma_start(wqm[1][:, 0:8, :], h["wqkvT"][1][:, 0:8, :])
            nc.sync.dma_start(xb0[:, 0:4, :], h["xT"][0][:, 0:4, :])
            nc.sync.dma_start(xb0[:, 4:8, :], h["xT"][0][:, 4:8, :])
            for m in range(2, NM1):
                q = nc.scalar if m % 2 == 0 else nc.gpsimd
                q.dma_start(wqm[m][:, 0:8, :], h["wqkvT"][m][:, 0:8, :])
            for ck in range(1, 4):
                tsl = slice(ck * 8, (ck + 1) * 8)
                for m in range(NM1):
                    q = nc.scalar if m % 2 == 0 else nc.gpsimd
                    q.dma_start(wqm[m][:, tsl, :], h["wqkvT"][m][:, tsl, :])
                nc.sync.dma_start(xb0[:, tsl, :], h["xT"][0][:, tsl, :])
            nc.scalar.dma_start(tri, h["tri"])
            nc.scalar.dma_start(idbf, h["idbf"])
            nc.scalar.dma_start(cos_t, h["cos"])
            nc.scalar.dma_start(sin_t, h["sin"])

            xb_hold = {0: xb0}

            def p1_copy(m, dst, ps):
                if m % 2 == 0:
                    nc.scalar.copy(dst, ps)
                else:
                    nc.vector.tensor_copy(dst, ps)

            # block 0: all 6 output tiles accumulate in parallel so each
            # arriving x/w chunk feeds 48 matmuls (keeps PE demand under
            # the DMA rate during the cold start)
            ps6 = [
                pp.tile([128, TB], F32, tag="op", bufs=4, name=f"ps6_{i}")
                for i in range(4)
            ]
            ps6 += [
                pp.tile([128, 1024], F32, tag="sc2", bufs=2, name=f"ps6_{i}")[
                    :, 0:TB
                ]
                for i in (4, 5)
            ]
            for ck in range(4):
                for m in range(NM1):
                    for t in range(ck * 8, (ck + 1) * 8):
                        nc.tensor.matmul(
                            ps6[m],
                            lhsT=wqm[m][:, t, :],
                            rhs=xb0[:, t, :],
                            start=(t == 0),
                            stop=(t == ET1 - 1),
                        )
            for m in range(NM1):
                p1_copy(m, qkv[m][:, 0:TB], ps6[m])

            def p1_block(n, m):
                if n not in xb_hold:
                    xb_n = p1.tile([128, ET1, TB], BF16, tag="xb", bufs=2)
                    nc.sync.dma_start(xb_n[:, 0:16, :], h["xT"][n][:, 0:16, :])
                    nc.gpsimd.dma_start(
                        xb_n[:, 16:32, :], h["xT"][n][:, 16:32, :]
                    )
                    xb_hold.clear()
                    xb_hold[n] = xb_n
                xb = xb_hold[n]
                ps = pp.tile([128, TB], F32, tag="op", bufs=4)
                for t in range(ET1):
                    nc.tensor.matmul(
                        ps,
                        lhsT=wqm[m][:, t, :],
                        rhs=xb[:, t, :],
                        start=(t == 0),
                        stop=(t == ET1 - 1),
                    )
                p1_copy(m, qkv[m][:, n * TB : (n + 1) * TB], ps)

            # b0-dependent DVE work (v-transpose, all five b0 RoPE slices)
            # spread between block-4's m-steps so no engine sees a burst
            hooks4 = {
                0: lambda: [vaug_one(0, vt) for vt in range(8)],
                1: lambda: [vaug_one(0, vt) for vt in range(8, 16)],
                2: lambda: [rope_chunk(0, 4, c) for c in range(4)],
                3: lambda: [rope_chunk(0, 0, c) for c in range(4)]
                + [rope_chunk(0, 1, c) for c in range(2)],
                4: lambda: [rope_chunk(0, 1, c) for c in range(2, 4)]
                + [rope_chunk(0, 2, c) for c in range(4)],
                5: lambda: [rope_chunk(0, 3, c) for c in range(4)],
            }
            for n in range(1, 5):
                for m in range(NM1):
                    p1_block(n, m)
                    if n == 4:
                        hooks4[m]()

            # leftover phase-1 blocks, b1 RoPE chunks, and b1 v-transposes
            # sequenced so every item's inputs precede it; pumped into the
            # 4-way-interleaved b0 heads (each head's dependency stalls
            # fill with the other heads' independent work)
            def P(n, m):
                pe_q.append(lambda: p1_block(n, m))

            def R(hs, c):
                pe_q.append(lambda: rope_chunk(1, hs, c))

            def V(vt):
                pe_q.append(lambda: vaug_one(1, vt))

            for m in (4, 5, 0, 1, 2, 3):
                P(5, m)
            for hs in (4, 0, 1, 2, 3):
                R(hs, 0)
                R(hs, 1)
            for vt in range(8):
                V(vt)
            for m in (4, 5, 0, 1, 2, 3):
                P(6, m)
            for hs in (4, 0, 1, 2, 3):
                R(hs, 2)
            for vt in range(8, 12):
                V(vt)
            for m in (4, 5, 0, 1, 2, 3):
                P(7, m)
            for hs in (4, 0, 1, 2, 3):
                R(hs, 3)
            for vt in range(12, 16):
                V(vt)
            set_rate(0.7)
            for j in range(4):
                for hh in range(4):
                    head_j(0, hh, j, u_cb=pump)
            while pe_q:
                pe_q.popleft()()
        # ---------------- late pool: w_out, store staging ---------------
        _p2cm = tc.tile_pool(name="p2", bufs=1)
        p2 = _p2cm.__enter__()
        for ob in range(8):
            w_ = p2.tile([128, 4, 512], BF16, tag=f"wo{ob}", name=f"wo{ob}")
            nc.sync.dma_start(w_, h["woutT"][ob])
            wo[ob] = w_



        # ---------------- phase 3 chunk emitters -------------------------
        bst_cur = [None]

        def p3_group(tt, g):
            if g == 0:
                bst_new = p2.tile([128, EMBED], BF16, tag="ost", bufs=2)
                bst_cur[0] = bst_new
            bst = bst_cur[0]
            psA = pp.tile([128, 512], F32, tag="op", bufs=4)
            psB = pp.tile([128, 512], F32, tag="op", bufs=4)
            for et in range(4):
                lt = aoT[et][:, tt * 128 : (tt + 1) * 128]
                nc.tensor.matmul(
                    psA, lhsT=lt, rhs=wo[2 * g][:, et, :],
                    start=(et == 0), stop=(et == 3),
                )
                nc.tensor.matmul(
                    psB, lhsT=lt, rhs=wo[2 * g + 1][:, et, :],
                    start=(et == 0), stop=(et == 3),
                )
            nc.scalar.copy(bst[:, (2 * g) * 512 : (2 * g + 1) * 512], psA)
            nc.vector.tensor_copy(
                bst[:, (2 * g + 1) * 512 : (2 * g + 2) * 512], psB
            )
            # half-row stores on alternating queues keep the tail short;
            # the last rows go out as quarters on three queues so the
            # final drain only waits on ~256KB
            rows = slice(tt * 128, (tt + 1) * 128)
            if g == 1:
                if tt >= 30:
                    nc.sync.dma_start(h["out"][rows, 0:1024], bst[:, 0:1024])
                    nc.scalar.dma_start(
                        h["out"][rows, 1024:2048], bst[:, 1024:2048]
                    )
                elif tt >= 24:
                    # keep the gpsimd ring empty near the end so its DGE
                    # drain overlaps remaining compute
                    q = nc.sync if tt % 2 == 0 else nc.scalar
                    q.dma_start(h["out"][rows, 0:2048], bst[:, 0:2048])
                else:
                    q = nc.gpsimd if tt % 2 == 0 else nc.sync
                    q.dma_start(h["out"][rows, 0:2048], bst[:, 0:2048])
            elif g == 3:
                if tt >= 30:
                    nc.sync.dma_start(
                        h["out"][rows, 2048:3072], bst[:, 2048:3072]
                    )
                    nc.scalar.dma_start(
                        h["out"][rows, 3072:4096], bst[:, 3072:4096]
                    )
                elif tt >= 24:
                    q = nc.scalar if tt % 2 == 0 else nc.sync
                    q.dma_start(h["out"][rows, 2048:4096], bst[:, 2048:4096])
                else:
                    q = nc.sync if tt % 2 == 0 else nc.gpsimd
                    q.dma_start(h["out"][rows, 2048:4096], bst[:, 2048:4096])

        # b1 attention, 4-way interleaved, with phase-3 b0-chunks pumped
        # per u-step; b1 chunks join the pool as soon as all four heads
        # complete each 512-token j-block
        pe_q.extend(
            lambda t_=tt, g_=g: p3_group(t_, g_)
            for tt in range(16) for g in range(4)
        )
        set_rate(0.8)
        for j in range(4):
            for hh in range(4):
                head_j(1, hh, j, u_cb=pump)
            pe_q.extend(
                lambda t_=tt, g_=g: p3_group(t_, g_)
                for tt in range(16 + 4 * j, 20 + 4 * j) for g in range(4)
            )
            if j == 0:
                set_rate(1.2)
        while pe_q:
            pe_q.popleft()()
        _p2cm.__exit__(None, None, None)


def _declare(nc):
    h = {}
    h["xT"] = nc.dram_tensor("xT", [NB, 128, ET1, TB], BF16, kind="ExternalInput").ap()
    h["wqkvT"] = nc.dram_tensor("wqkvT", [NM1, 128, ET1, 128], BF16, kind="ExternalInput").ap()
    h["woutT"] = nc.dram_tensor("woutT", [8, 128, 4, 512], BF16, kind="ExternalInput").ap()
    h["cos"] = nc.dram_tensor("cosT", [128, S], BF16, kind="ExternalInput").ap()
    h["sin"] = nc.dram_tensor("sinT", [128, S], BF16, kind="ExternalInput").ap()
    h["tri"] = nc.dram_tensor("tri", [128, 128], BF16, kind="ExternalInput").ap()
    h["idbf"] = nc.dram_tensor("idbf", [128, 128], BF16, kind="ExternalInput").ap()
    h["out"] = nc.dram_tensor("out", [TOK, EMBED], BF16, kind="ExternalOutput").ap()
    return h


_CACHE = {}


def _get_nc():
    if "nc" not in _CACHE:
        nc = bacc.Bacc(None, target_bir_lowering=False, debug=False)
        h = _declare(nc)
        with tile.TileContext(nc) as tc:
            _emit(nc, tc, h)
        nc.compile()
        _CACHE["nc"] = nc
    return _CACHE["nc"]


def _prep_in_maps(x, w_qkv, w_out):
    x = np.asarray(x, dtype=np.float32)
    w_qkv = np.asarray(w_qkv, dtype=np.float32)
    w_out = np.asarray(w_out, dtype=np.float32)

    xT = x.reshape(TOK, EMBED).T  # [E, TOK]
    xT = np.ascontiguousarray(
        xT.reshape(ET1, 128, NB, TB).transpose(2, 1, 0, 3)
    ).astype(NPBF16)  # [n, p, t, c]

    # RoPE tables, half-layout; sin sign-folded: rows 0:64 = -sin (pairs
    # x1*c - x2*s), rows 64:128 = +sin (x2*c + x1*s)
    invf = ROPE_BASE ** (-np.arange(0, HEAD_DIM, 2, dtype=np.float32) / HEAD_DIM)
    ang = invf[:, None].astype(np.float64) * np.arange(S, dtype=np.float64)[None, :]
    cosT = np.concatenate([np.cos(ang), np.cos(ang)], axis=0).astype(NPBF16)
    sinT = np.concatenate([-np.sin(ang), np.sin(ang)], axis=0).astype(NPBF16)

    tri = np.triu(np.ones((128, 128), dtype=np.float32)).astype(NPBF16)
    idbf = np.eye(128, dtype=np.float32).astype(NPBF16)

    # interleaved -> half-layout permutation of the head dim, applied to the
    # q/k rows of the weight (scores are invariant to a shared permutation)
    perm = np.concatenate([np.arange(0, 128, 2), np.arange(1, 128, 2)])

    in_maps = []
    for c in range(NCORES):
        ws = w_qkv[c * ODIM : (c + 1) * ODIM].copy()
        for hb in range(5):  # 4 q heads + k
            ws[hb * 128 : (hb + 1) * 128] = ws[hb * 128 : (hb + 1) * 128][perm]
        wqkvT = ws.T.reshape(ET1, 128, NM1, 128).transpose(2, 1, 0, 3)
        wqkvT = np.ascontiguousarray(wqkvT).astype(NPBF16)  # [m, p, t, d]
        woutT = w_out[:, c * 512 : (c + 1) * 512].T  # [512, E]
        woutT = np.ascontiguousarray(
            woutT.reshape(4, 128, 8, 512).transpose(2, 1, 0, 3)
        ).astype(NPBF16)  # [ob, p, et, o]
        in_maps.append(
            {
                "xT": xT,
                "wqkvT": wqkvT,
                "woutT": woutT,
                "cosT": cosT,
                "sinT": sinT,
                "tri": tri,
                "idbf": idbf,
            }
        )
    return in_maps


def _run(inputs, trace=False):
    nc = _get_nc()
    in_maps = _prep_in_maps(inputs["x"], inputs["w_qkv"], inputs["w_out"])
    res = run_bass_kernel_spmd(nc, in_maps, list(range(NCORES)), trace=trace)
    acc = np.zeros((TOK, EMBED), dtype=np.float32)
    for r in res.results:
        acc += np.asarray(r["out"]).astype(np.float32)
    out = acc.reshape(B, S, EMBED)
    return out, res.exec_time_ns


def kernel(**inputs):
    out, _ = _run(inputs, trace=False)
    return out
